# revision 68
# baseline (speedup 1.0000x reference)
# HEPOS cross-attention (strided per-head K/V) on 8 Trainium2 NeuronCores.
#
# Reference computation (per head h, stride s = STRIDE_LIST[h]):
#   Q = x @ Wq.T + bq ; K = e @ Wk.T + bk ; V = e @ Wv.T + bv
#   out_h = softmax(Q_h @ K_h[::s].T / 8) @ V_h[::s]
#   out   = concat_h(out_h) @ Wo.T + bo
#
# Sharding: 64 (batch, head) units over 8 cores. Core c owns head group
# g = c % 4 (heads 4g..4g+3, strides [1,2,4,8]) and batch pair [0,1]
# (c < 4) or [2,3] (c >= 4). Each core computes its heads' contribution
# to out; the host sums the four partials per batch and adds bo.
#
# On-device design (v3):
#  * Heads are processed as two stride PAIRS (sA, 2*sA): (1,2) and (4,8).
#    Head A of a pair lives on SBUF partitions 0-63, head B on 64-127.
#  * Q/K/V projections use the weight matrix as the matmul stationary with
#    both heads packed into the 128 stationary columns (full PE width).
#    K^T/V^T stream the "union" encoder columns (stride sA); head B rows
#    are valid at even union columns and are compacted on evacuation.
#    The stride-4 union for pair (4,8) is pre-packed by the host (eT4).
#  * V^T ([hd, keys]) is flipped to AV orientation ([keys, hd]) with ONE
#    DMA XBAR transpose per (block, head) (3D output access pattern) -
#    zero PE cost, one sync-engine dispatch each.
#  * Scores are computed transposed ([keys, T]); head B's score matmuls
#    use partitions 64-127 (PE row-tile T8) and overlap head A's (T0).
#  * AV accumulates into PSUM tiles resident across all encoder blocks of
#    a (batch, pair); the softmax denominator falls out of a ones-column
#    appended to the V stationary.
#  * Scores of chunk k+1 are issued before AV of chunk k so the PE never
#    waits on the scalar engine's exp.
#  * All DRAM->SBUF loads are single merged DMAs ([128, ndc, *] access
#    patterns); PSUM score tiles are bank-pair wide ([128, 2*tt]) so exp
#    and evacuations run as one instruction per tile.
#
# v4 (engine-FIFO / HAM-warmth pass; v3 measured ~150us of its 385us span
# at the 1.2 GHz throttled PE clock, re-triggered at every pair boundary):
#  * softmax normalize rebuilt: packed scalar+vector PSUM evacuations, one
#    reciprocal_approx_fast, K=2 selector-matmul partition broadcast into
#    the freed AV PSUM bank, one [128,tt] multiply per nt. No GpSimd, no
#    3.3us DVE reciprocal, vector-queue occupancy ~2.5us (was ~9us).
#  * exp merged to one ACT instruction per chunk ([128, 2tt]).
#  * out-proj evacuations alternate scalar/vector; store DMAs dispatch from
#    the gpsimd queue (sync queue head-of-line blocking broke et prefetch).
#  * startup: bias DMA early, warmup 40 x N=256 sized to the initial DMA
#    window, Q-proj double-buffered across the sc/kv PSUM pools.

import os
import sys

import ml_dtypes
import numpy as np

BF16 = ml_dtypes.bfloat16

for _p in ("/opt/trn_rl_repo", "/root/.axon_site/_ro/trn_rl_repo"):
    if os.path.isdir(_p) and _p not in sys.path:
        sys.path.insert(0, _p)

import concourse.bass as bass  # noqa: E402
import concourse.tile as tile  # noqa: E402
from concourse import bacc, mybir  # noqa: E402
from concourse import bass_utils  # noqa: E402

F32 = mybir.dt.float32
MM_DT = mybir.dt.bfloat16  # matmul operand dtype: full PE rate, half DMA
AF = mybir.ActivationFunctionType

D_MODEL = 1024
NUM_HEADS = 16
HEAD_DIM = 64
STRIDE_LIST = [1, 2, 4, 8] * 4
B, T, S = 4, 1024, 4096
N_CORES = 8

FULL_CFG = dict(
    nb=2,  # batches per core
    T=T,
    S=S,
    D=D_MODEL,
    strides=(1, 2, 4, 8),  # per-core head strides; pairs (s0,s1),(s2,s3)
    hd=HEAD_DIM,
    blk=1024,  # encoder S-block (stride-1 columns) per iteration
    tt=512,  # T tile (PSUM free-dim limit for fp32)
)

FR = MM_DT
WHOLE_BLOCK_TRANSPOSE = True


def _mm(nc, out, lhsT, rhs, start, stop):
    nc.tensor.matmul(out, lhsT, rhs, start=start, stop=stop)


def build_program(cfg):
    """Build the per-core Bass/Tile program (same program on all cores)."""
    nb, Tl, Sl, Dl = cfg["nb"], cfg["T"], cfg["S"], cfg["D"]
    strides, hd = cfg["strides"], cfg["hd"]
    assert strides[1] == 2 * strides[0] and strides[3] == 2 * strides[2]
    HP = 4 * hd  # packed head rows (256)
    s4 = strides[2]

    nc = bacc.Bacc(
        "TRN2",
        target_bir_lowering=False,
        debug=False,
        enable_asserts=False,
        num_devices=N_CORES,
    )

    # All inputs are HOST-PREPACKED to partition-major [128, ...] layouts so
    # every device DMA is a contiguous 2D slice (128 descriptors). The old
    # "(c p) u -> p c u" gather patterns cost ~1024 descriptors = 2-5.5us of
    # serial sync-engine dispatch PER LOAD, which dominated startup and
    # delayed et prefetches mid-stream.
    ndc_ = Dl // 128
    xT = nc.dram_tensor(
        "xT", [128, nb * ndc_ * Tl], MM_DT, kind="ExternalInput"
    ).ap()
    eT = nc.dram_tensor(
        "eT", [128, nb * ndc_ * Sl], MM_DT, kind="ExternalInput"
    ).ap()
    eT4 = nc.dram_tensor(
        "eT4", [128, nb * ndc_ * (Sl // s4)], MM_DT, kind="ExternalInput"
    ).ap()
    wqT = nc.dram_tensor("wqT", [128, ndc_ * HP], MM_DT, kind="ExternalInput").ap()
    wkT = nc.dram_tensor("wkT", [128, ndc_ * HP], MM_DT, kind="ExternalInput").ap()
    wvT = nc.dram_tensor("wvT", [128, ndc_ * HP], MM_DT, kind="ExternalInput").ap()
    woT = nc.dram_tensor("woT", [128, 2 * Dl], MM_DT, kind="ExternalInput").ap()
    biases = nc.dram_tensor("biases", [128, 6], F32, kind="ExternalInput").ap()
    out = nc.dram_tensor("partial", [nb * Tl, Dl], MM_DT, kind="ExternalOutput").ap()

    with tile.TileContext(nc) as tc:
        _build_tile(tc, cfg, xT, eT, eT4, wqT, wkT, wvT, woT, biases, out)

    nc.compile()
    return nc


def _build_tile(tc, cfg, xT, eT, eT4, wqT, wkT, wvT, woT, biases, out):
    nc = tc.nc
    nb, Tl, Sl, Dl = cfg["nb"], cfg["T"], cfg["S"], cfg["D"]
    strides, hd = cfg["strides"], cfg["hd"]
    blk, tt = cfg["blk"], cfg["tt"]
    ndc = Dl // 128
    nblk = Sl // blk
    ntt = Tl // tt
    assert ntt == 2, "wide PSUM tiles assume T == 2*tt"
    HP = 4 * hd
    scale = 1.0 / float(np.sqrt(hd))

    from contextlib import ExitStack

    with ExitStack() as ctx:
        wpool = ctx.enter_context(tc.tile_pool(name="weights", bufs=1))
        qtpool = ctx.enter_context(tc.tile_pool(name="qt", bufs=1))
        etpool = ctx.enter_context(tc.tile_pool(name="et", bufs=3))
        ktpool = ctx.enter_context(tc.tile_pool(name="kt", bufs=3))
        vtpool = ctx.enter_context(tc.tile_pool(name="vtT", bufs=2))
        vpool = ctx.enter_context(tc.tile_pool(name="v", bufs=3))
        ppool = ctx.enter_context(tc.tile_pool(name="p", bufs=4))
        npool = ctx.enter_context(tc.tile_pool(name="norm", bufs=2))
        otpool = ctx.enter_context(tc.tile_pool(name="ot", bufs=2))
        obpool = ctx.enter_context(tc.tile_pool(name="outs", bufs=3))
        # PSUM: ONE unified single-bank ring ([128, tt] fp32, bufs=4 -> 4
        # banks) shared by scores A/B, K/V projections, Q and out
        # projections. Each allocation only waits on the readers of the
        # allocation FOUR back, giving the score pipeline ~2 chunks of
        # slack over the exp stream - enough that transient ACT hiccups
        # never micro-stall the PE (micro-stalls oscillate the HAM clock
        # gate down to 1.2 GHz). av holds the 4 single-bank accumulators
        # -> 8 banks total.
        ps_ps = ctx.enter_context(tc.tile_pool(name="ps", bufs=4, space="PSUM"))
        av_ps = ctx.enter_context(tc.tile_pool(name="av_ps", bufs=1, space="PSUM"))

        def ps_tile():
            return ps_ps.tile([128, tt], F32, tag="ps", name="ps_psum")

        # ---- weights into SBUF (one DMA per tensor) ----
        wq_sb = wpool.tile([128, ndc * HP], FR, tag="wq", name="wq_sb")
        wk_sb = wpool.tile([128, ndc * HP], FR, tag="wk", name="wk_sb")
        wv_sb = wpool.tile([128, ndc * HP], FR, tag="wv", name="wv_sb")
        wo_sb = wpool.tile([128, 2 * Dl], FR, tag="wo", name="wo_sb")
        bias_sb = wpool.tile([128, 6], F32, tag="bias", name="bias_sb")
        # selector for the reciprocal partition-broadcast matmul:
        # rb[j, t] = rcp4[32 * (2nt + j // hd), t] via rb = sel.T @ rcp4-rows.
        # K=33 dense with live rows {0, 32} (legal start partitions); the
        # zero rows cancel den4's filler rows.
        sel_sb = wpool.tile([33, 128], FR, tag="sel", name="sel_sb")
        # rows {0, 32} = heads (legal start partitions), column halves = nt;
        # persistent + memset once so the filler rows stay initialized for
        # the whole-tile reciprocal.
        den4 = wpool.tile([33, 2 * tt], F32, tag="den4", name="den4")

        def wslice(wsb, dc, p):
            return wsb[:, dc * HP + p * 128 : dc * HP + (p + 1) * 128]

        # encoder block list + DMA helper (defined early so the first
        # block's load can be interleaved with the weight loads).
        # Blocks are uniform in UNION columns (blk per block) so pair (4,8)
        # gets one full-sized block instead of four tiny ones.
        def block_params(p):
            sA = strides[2 * p]
            Scols = Sl // sA
            return dict(
                src=eT if p == 0 else eT4,
                Scols=Scols,
                nblk_p=max(1, Scols // blk),
            )

        blocks = [
            (b, p, ib)
            for b in range(nb)
            for p in range(2)
            for ib in range(block_params(p)["nblk_p"])
        ]

        def block_ublk(p, ib):
            bp = block_params(p)
            return min(blk, bp["Scols"] - ib * blk)

        def emit_et_dma(b, p, ib):
            bp = block_params(p)
            ublk = block_ublk(p, ib)
            et = etpool.tile([128, ndc * ublk], FR, tag="et", name="et_t")
            base = (b * bp["nblk_p"] + ib) * ndc * ublk
            nc.sync.dma_start(out=et, in_=bp["src"][:, base : base + ndc * ublk])
            return et

        # ---- startup DMAs, ordered by first use: Q(b0) needs wq+xt0+bias,
        # then K0/V0 need wk/et0/wv, then et1 (phase_a(1) filler), then xt1
        # (deferred Q(b1) fillers), then wo (first out-proj, much later).
        # xt0 is split in dc-halves so Q(b0,p0)'s first dc-outer matmul
        # groups start after half the transfer.
        xpool = ctx.enter_context(tc.tile_pool(name="xt", bufs=1))
        xts = {}
        for b in range(nb):
            xts[b] = xpool.tile([128, ndc * Tl], FR, tag=f"xt{b}", name="xt")
        nc.sync.dma_start(out=wq_sb, in_=wqT)
        xh = (ndc // 2) * Tl
        nc.sync.dma_start(out=xts[0][:, 0:xh], in_=xT[:, 0:xh])
        nc.sync.dma_start(out=xts[0][:, xh : ndc * Tl], in_=xT[:, xh : ndc * Tl])
        nc.sync.dma_start(out=bias_sb, in_=biases)
        nc.sync.dma_start(out=wk_sb, in_=wkT)
        et_next = emit_et_dma(*blocks[0])
        nc.sync.dma_start(out=wv_sb, in_=wvT)
        et_next1 = emit_et_dma(*blocks[1]) if len(blocks) > 1 else None
        for b in range(1, nb):
            nc.sync.dma_start(
                out=xts[b],
                in_=xT[:, b * ndc * Tl : (b + 1) * ndc * Tl],
            )
        nc.sync.dma_start(out=wo_sb, in_=woT)
        nc.vector.memset(sel_sb, 0.0)
        nc.vector.memset(sel_sb[0:1, 0:hd], 1.0)
        nc.vector.memset(sel_sb[32:33, hd:128], 1.0)
        nc.vector.memset(den4, 1.0)

        # PE warm-up: dependency-free matmuls on a zeroed tile keep the
        # PE busy while the first DMAs land (~7us), so the HAM clock gate
        # opens (1.2 -> 2.4 GHz) before the real matmul stream begins.
        # 36 x N=256 at cold clock ~= 7.5us - sized to the DMA window.
        warm = wpool.tile([128, tt], FR, tag="warm", name="warm")
        nc.vector.memset(warm, 0.0)
        wps = ps_tile()
        for _ in range(36):
            _mm(
                nc, wps[:, 0:256], warm[:, 0:128], warm[:, 0:256],
                start=True, stop=True,
            )

        # ---- phase 1: Q^T = (x @ Wq.T + bq)^T, head pairs on partitions.
        # Only batch 0 is emitted up front; later batches become filler
        # items inside phase_b so they never block the tensor FIFO waiting
        # on their xt DMA.
        qt_sb = {}  # (b, pair) -> [128, T] tile

        def q_item(b, p):
            # dc-outer so the first matmul groups only need the first
            # dc-half of the xt DMA, and each stationary serves both nt.
            qt = qtpool.tile([128, Tl], FR, tag=f"qt{b}{p}", name="qt")
            qt_sb[(b, p)] = qt
            ps = [ps_tile() for _ in range(ntt)]
            for dc in range(ndc):
                for nt in range(ntt):
                    _mm(
                        nc,
                        ps[nt],
                        wslice(wq_sb, dc, p),
                        xts[b][:, dc * Tl + nt * tt : dc * Tl + (nt + 1) * tt],
                        start=(dc == 0),
                        stop=(dc == ndc - 1),
                    )
            for nt in range(ntt):
                nc.scalar.activation(
                    qt[:, nt * tt : (nt + 1) * tt],
                    ps[nt],
                    AF.Identity,
                    bias=bias_sb[:, p : p + 1],
                )

        for p in range(2):
            q_item(0, p)
        # bridge the Q-proj -> K-proj handoff: K0 waits on the et0 DMA
        # (~2.5us after Q drains); keep the PE busy so HAM stays warm.
        for _ in range(3):
            brg = ps_tile()
            for _ in range(4):
                _mm(
                    nc, brg[:, 0:256], warm[:, 0:128], warm[:, 0:256],
                    start=True, stop=True,
                )

        # ---- phase 2: attention per (batch, pair), out proj per batch ----
        # The per-block work is split into phase A (K^T/V^T projection,
        # evacuation, V transposes, next-block encoder DMA) and phase B
        # (scores/exp/AV chunk loop), software-pipelined one block deep:
        #   pA(0) pA(1) pB(0) pA(2) pB(1) ... pA(n-1) pB(n-3) pB(n-2) pB(n-1)
        # so V transposes are dispatched a full block before their AV
        # consumes them and the normalize chain never blocks evacuations.
        # AV emission inside phase B additionally lags scores by two chunk
        # steps so the PE never waits on the scalar engine's exp.
        assert Dl <= 2 * tt
        ot_sb = {}
        avp_live = {}
        blk_state = {}
        pending = []  # (age, avp, vt, pt, h, first, last)

        def flush_pending(min_age=2):
            keep = []
            for age, avp, vt, pt, h, first, last in pending:
                if age >= min_age:
                    for nt in range(ntt):
                        _mm(
                            nc,
                            avp[(h, nt)],
                            vt,
                            pt[:, nt * tt : (nt + 1) * tt],
                            start=first,
                            stop=last,
                        )
                else:
                    keep.append((age + 1, avp, vt, pt, h, first, last))
            pending[:] = keep

        def emit_norm(b, p):
            """Normalize the AV accumulators into ot. Designed to keep every
            engine FIFO short at pair boundaries (a clogged vector queue here
            stalls the next block's PSUM evacuations, idles the PE > 3.4us and
            drops the HAM clock gate to 1.2 GHz - the dominant cost in v3):
              * PSUM->SBUF evacuations split across scalar (h0) and vector
                (h1) queues, numerators packed on partitions 0:64 / 64:128.
              * reciprocal_approx_fast (one custom-DVE op, ~5x faster than
                reciprocal) on a [2, 2tt] tile holding all four denominators.
              * partition-broadcast of the reciprocals via a tiny K=2
                selector MATMUL into the just-freed AV PSUM bank (replaces
                two 1.1us GpSimd broadcast ping-pongs per (h,nt)).
              * one [128, tt] vector multiply per nt writes ot."""
            avp = avp_live.pop((b, p))
            rcp4 = npool.tile([33, 2 * tt], F32, tag="rcp4", name="rcp4")
            rcpb = npool.tile([33, 2 * tt], FR, tag="rcpb", name="rcpb")
            packed = {}
            for nt in range(ntt):
                pk = npool.tile([128, tt], F32, tag=f"pk{nt}", name="packed")
                packed[nt] = pk
                nc.scalar.copy(pk[0:hd, :], avp[(0, nt)][0:hd, :])
                nc.vector.tensor_copy(pk[hd : 2 * hd, :], avp[(1, nt)][0:hd, :])
                # den row copies cost a full free-dim pass each (DVE/ACT are
                # free-dim serial) - split them across the two engines
                nc.scalar.copy(
                    den4[0:1, nt * tt : (nt + 1) * tt],
                    avp[(0, nt)][hd : hd + 1, :],
                )
                nc.vector.tensor_copy(
                    den4[32:33, nt * tt : (nt + 1) * tt],
                    avp[(1, nt)][hd : hd + 1, :],
                )
            nc.vector.reciprocal_approx_fast(rcp4, den4)
            nc.scalar.copy(rcpb, rcp4)  # bf16 for the cheap selector matmul

            def part2():
                # the PE/vector half of the normalize, deferred a full block
                # (via the stash) so rcpb is long done when these drain as
                # filler items between score chunks.
                ot = otpool.tile([128, Tl], FR, tag=f"ot{p}", name="ot")
                ot_sb[(b, p)] = ot
                for nt in range(ntt):
                    rb = ps_tile()
                    _mm(
                        nc,
                        rb,
                        sel_sb,
                        rcpb[0:33, nt * tt : (nt + 1) * tt],
                        start=True,
                        stop=True,
                    )
                    nc.vector.tensor_mul(
                        ot[:, nt * tt : (nt + 1) * tt], packed[nt], rb
                    )

            return part2

        def out_proj_items(b):
            """One self-contained filler item per 128-row output tile:
            [ps alloc, 4 MMs, evacuation, store DMA]. Items are drained one
            per chunk inside phase_b so these exp-independent matmuls fill
            the PE's exp-wait bubbles instead of lumping at block edges."""

            def item(tc_i):
                ob = obpool.tile([128, Dl], FR, tag="ob", name="ob")
                for j in range(0, Dl, tt):
                    dw = min(tt, Dl - j)
                    ops = ps_tile()
                    for p in range(2):
                        _mm(
                            nc,
                            ops[:, 0:dw],
                            ot_sb[(b, p)][:, tc_i * 128 : (tc_i + 1) * 128],
                            wo_sb[:, p * Dl + j : p * Dl + j + dw],
                            start=(p == 0),
                            stop=(p == 1),
                        )
                    # mid-stream: keep evacs off the scalar queue (it is
                    # saturated with exps). Tail batch: split scalar/vector
                    # (no exps left; vector alone serializes the tail) and
                    # store each half immediately so the final DMA drain
                    # overlaps the remaining matmuls instead of trailing.
                    if b == nb - 1:
                        if j == 0:
                            nc.scalar.copy(ob[:, j : j + dw], ops[:, 0:dw])
                        else:
                            nc.vector.tensor_copy(ob[:, j : j + dw], ops[:, 0:dw])
                        eng = nc.sync if tc_i % 2 == 1 else nc.gpsimd
                        eng.dma_start(
                            out=out[
                                b * Tl + tc_i * 128 : b * Tl + (tc_i + 1) * 128,
                                j : j + dw,
                            ],
                            in_=ob[:, j : j + dw],
                        )
                    else:
                        nc.vector.tensor_copy(ob[:, j : j + dw], ops[:, 0:dw])
                if b < nb - 1:
                    # the store DMA is dispatched from the (idle) gpsimd
                    # queue so its input-ready wait never head-of-line
                    # blocks the sync queue's et-prefetch / V-transpose
                    # dispatches.
                    nc.gpsimd.dma_start(
                        out=out[
                            b * Tl + tc_i * 128 : b * Tl + (tc_i + 1) * 128, :
                        ],
                        in_=ob,
                    )

            import functools

            return [functools.partial(item, tc_i) for tc_i in range(Tl // 128)]

        ets = {0: et_next}
        if et_next1 is not None:
            ets[1] = et_next1

        def phase_a_items(bi):
            """The per-block K/V projection as self-contained per-tt-half
            filler items ([ps alloc, MMs, evacuations]) plus the V
            transposes / next-block DMA. Drained between phase_b chunks so
            the PE's exp-gated bubbles absorb the projection stream."""
            b, p, ib = blocks[bi]
            ublk = block_ublk(p, ib)
            nA = ublk // 128
            nB = nA // 2
            st = {}

            def proj_half(kind, c0):
                cw = min(tt, ublk - c0)
                if kind == "k":
                    if c0 == 0:
                        st["ktA"] = ktpool.tile(
                            [64, ublk], FR, tag="ktA", name="kt_A"
                        )
                        st["ktB"] = ktpool.tile(
                            [128, ublk // 2], FR, tag="ktB", name="kt_B"
                        )
                    dst_A, dst_B = st["ktA"], st["ktB"]
                    wsb, bcol = wk_sb, 2 + p
                else:
                    if c0 == 0:
                        st["vtA"] = vtpool.tile(
                            [64, ublk], FR, tag="vtA", name="vtT_A"
                        )
                        st["vtB"] = vtpool.tile(
                            [128, ublk // 2], FR, tag="vtB", name="vtT_B"
                        )
                    dst_A, dst_B = st["vtA"], st["vtB"]
                    wsb, bcol = wv_sb, 4 + p
                pps = ps_tile()
                et = ets[bi]
                for dc in range(ndc):
                    _mm(
                        nc,
                        pps[:, 0:cw],
                        wslice(wsb, dc, p),
                        et[:, dc * ublk + c0 : dc * ublk + c0 + cw],
                        start=(dc == 0),
                        stop=(dc == ndc - 1),
                    )
                nc.vector.tensor_scalar_add(
                    dst_A[:, c0 : c0 + cw],
                    pps[0:64, 0:cw],
                    bias_sb[0:64, bcol : bcol + 1],
                )
                nc.vector.tensor_scalar_add(
                    dst_B[64:128, c0 // 2 : (c0 + cw) // 2],
                    pps[64:128, 0:cw:2],
                    bias_sb[64:128, bcol : bcol + 1],
                )

            v_c0s = list(range(0, ublk, tt))

            def last_v_half():
                proj_half("v", v_c0s[-1])
                ets.pop(bi)
                vtT_A, vtT_B = st["vtA"], st["vtB"]

                # prefetch the NEXT block's encoder tile before the transposes
                if bi + 1 < len(blocks) and bi + 1 not in ets:
                    ets[bi + 1] = emit_et_dma(*blocks[bi + 1])

                # V -> [keys, hd] via DMA XBAR transposes - zero PE cost.
                # Chunk pitch 80 elems (160B) keeps destinations 32B-aligned.
                VP = hd + 16
                vtA = vpool.tile([128, nA * VP], FR, tag="vA", name="vtA")
                vtA3 = vtA.rearrange("p (c f) -> p c f", c=nA)
                nc.vector.memset(vtA3[:, :, hd : hd + 1], 1.0)
                vtB = vpool.tile([128, nB * VP], FR, tag="vB", name="vtB")
                vtB3 = vtB.rearrange("p (c f) -> p c f", c=nB)
                nc.vector.memset(vtB3[:, :, hd : hd + 1], 1.0)
                nc.sync.dma_start(out=vtA3[:, :, 0:hd], in_=vtT_A, transpose=True)
                nc.sync.dma_start(
                    out=vtB3[:, :, 0:hd], in_=vtT_B[64:128, :], transpose=True
                )
                blk_state[bi] = (st["ktA"], st["ktB"], vtA, vtB, nA, nB)

            import functools

            items = [
                functools.partial(proj_half, "k", c0)
                for c0 in range(0, ublk, tt)
            ]
            items += [
                functools.partial(proj_half, "v", c0) for c0 in v_c0s[:-1]
            ]
            items.append(last_v_half)
            return items

        def phase_b(bi, urgent, low):
            b, p, ib = blocks[bi]
            # safety: finish this block's phase_a / this batch's q_item
            while bi not in blk_state or (b, p) not in qt_sb:
                urgent.popleft()()
            kt_A, kt_B, vtA, vtB, nA, nB = blk_state.pop(bi)
            nblk_p = block_params(p)["nblk_p"]
            VP = hd + 16
            if (b, p) not in avp_live:
                avp_live[(b, p)] = {
                    (h, nt): av_ps.tile(
                        [hd + 1, tt], F32, tag=f"av{h}{nt}", name="av_psum"
                    )
                    for h in range(2)
                    for nt in range(ntt)
                }
            avp = avp_live[(b, p)]

            for ck in range(nA):
                do_B = ck % 2 == 1
                ckb = ck // 2
                ptA = ppool.tile([128, Tl], FR, tag="pA", name="ptA")
                sa = [ps_tile() for _ in range(ntt)]
                if do_B:
                    ptB = ppool.tile([128, Tl], FR, tag="pB", name="ptB")
                    sb_ = [ps_tile() for _ in range(ntt)]
                for nt in range(ntt):
                    _mm(
                        nc,
                        sa[nt],
                        kt_A[:, ck * 128 : (ck + 1) * 128],
                        qt_sb[(b, p)][0:64, nt * tt : (nt + 1) * tt],
                        start=True,
                        stop=True,
                    )
                    if do_B:
                        _mm(
                            nc,
                            sb_[nt],
                            kt_B[64:128, ckb * 128 : (ckb + 1) * 128],
                            qt_sb[(b, p)][64:128, nt * tt : (nt + 1) * tt],
                            start=True,
                            stop=True,
                        )
                # per-nt exps: each releases its single score bank as soon
                # as it completes, keeping the 4-deep ring flowing.
                for nt in range(ntt):
                    nc.scalar.activation(
                        ptA[:, nt * tt : (nt + 1) * tt], sa[nt], AF.Exp,
                        scale=scale,
                    )
                if do_B:
                    for nt in range(ntt):
                        nc.scalar.activation(
                            ptB[:, nt * tt : (nt + 1) * tt], sb_[nt], AF.Exp,
                            scale=scale,
                        )
                flush_pending()
                pending.append(
                    (
                        0,
                        avp,
                        vtA[:, ck * VP : ck * VP + hd + 1],
                        ptA,
                        0,
                        ib == 0 and ck == 0,
                        ib == nblk_p - 1 and ck == nA - 1,
                    )
                )
                if do_B:
                    pending.append(
                        (
                            0,
                            avp,
                            vtB[:, ckb * VP : ckb * VP + hd + 1],
                            ptB,
                            1,
                            ib == 0 and ckb == 0,
                            ib == nblk_p - 1 and ckb == nB - 1,
                        )
                    )
                # drain filler items: urgent (K/V projections - the et
                # prefetch chain depends on their progress) one per chunk;
                # deferrable work (out-proj, norm part2) only on odd chunks
                # so a reserve survives into the batch's last blocks.
                if urgent:
                    urgent.popleft()()
                elif low and (ck % 2 == 1 or bi >= nbl_holder[0] - 2):
                    low.popleft()()

            if ib == nblk_p - 1:
                flush_pending(min_age=0)
                norm_todo.append((b, p))

        from collections import deque

        nbl = len(blocks)
        nbl_holder = [nbl]
        norm_todo = []
        urgent = deque()
        low = deque()

        def dummy_item():
            # dependency-free warm-keeper matmuls: bridge the sparse
            # exp-paced stretches of the last blocks / final normalize so
            # the HAM activity window never re-throttles the PE to 1.2 GHz.
            dp = ps_tile()
            for _ in range(4):
                _mm(
                    nc, dp[:, 0:256], warm[:, 0:128], warm[:, 0:256],
                    start=True, stop=True,
                )
        for it in phase_a_items(0):
            it()
        if nbl > 1:
            urgent.extend(phase_a_items(1))
        for b in range(1, nb):
            urgent.extend(
                (lambda b_, p_: lambda: q_item(b_, p_))(b, p) for p in range(2)
            )
        stash = []  # norm part2 items, delayed one block so their inputs
        # (the pair-end reciprocal chain) are long done when they drain
        for i in range(nbl):
            if i + 2 < nbl:
                urgent.extend(phase_a_items(i + 2))
            b_i, p_i, ib_i = blocks[i]
            if b_i > 0 and p_i == 0 and ib_i == 0:
                # previous batch's out projection, deferred past its norm
                # (its emission must follow the stashed norm part2)
                low.extend(stash)
                stash.clear()
                low.extend(out_proj_items(b_i - 1))
            if i >= nbl - 2:
                low.extend([dummy_item] * 6)
            # norm part2 after the dummies: its rb matmul waits on the
            # pair-end reciprocal/cast chain, so give it extra runway
            low.extend(stash)
            stash.clear()
            phase_b(i, urgent, low)
            while norm_todo:
                nb_, np_ = norm_todo.pop(0)
                stash.append(emit_norm(nb_, np_))
        low.extend([dummy_item] * 10)
        low.extend(stash)
        stash.clear()
        while urgent:
            urgent.popleft()()
        while low:
            low.popleft()()
        for it in out_proj_items(nb - 1):
            it()

# ---------------------------------------------------------------------------
# Host-side sharding / gathering
# ---------------------------------------------------------------------------


def _core_map():
    """core -> (batches, heads)"""
    m = {}
    for c in range(N_CORES):
        g = c % 4
        bs = [0, 1] if c < 4 else [2, 3]
        hs = [4 * g + i for i in range(4)]
        m[c] = (bs, hs)
    return m


def pack_pd(a2d):
    """[N, D] -> [128, ndc * N] partition-major: out[p, dc*N + n] =
    a2d[n, dc*128 + p]. Makes the device DMA a contiguous 2D slice."""
    N, D = a2d.shape
    ndc = D // 128
    return (
        a2d.reshape(N, ndc, 128).transpose(2, 1, 0).reshape(128, ndc * N)
    )


def pack_blocks(e_b, blk, cfg):
    """Per-batch encoder rows [nb, Scols, D] -> [128, nb*nblk*ndc*ublk]
    with each (b, ib) block pack_pd'd and laid out contiguously."""
    nbb, Scols, D = e_b.shape
    nblk = max(1, Scols // blk)
    ublk = Scols // nblk
    parts = [
        pack_pd(e_b[b, ib * ublk : (ib + 1) * ublk])
        for b in range(nbb)
        for ib in range(nblk)
    ]
    return np.concatenate(parts, axis=1)


def shard_inputs(inputs, cfg):
    x = np.asarray(inputs["decoder_input"], np.float32)
    e = np.asarray(inputs["encoder_output"], np.float32)
    Wq = np.asarray(inputs["Wq"], np.float32)
    Wk = np.asarray(inputs["Wk"], np.float32)
    Wv = np.asarray(inputs["Wv"], np.float32)
    Wo = np.asarray(inputs["Wo"], np.float32)
    bq = np.asarray(inputs["bq"], np.float32)
    bk = np.asarray(inputs["bk"], np.float32)
    bv = np.asarray(inputs["bv"], np.float32)
    hd = cfg["hd"]
    s4 = cfg["strides"][2]
    blk = cfg["blk"]
    in_maps = []
    for c, (bs, hs) in _core_map().items():
        rows = np.concatenate([np.arange(h * hd, (h + 1) * hd) for h in hs])
        xb = x[bs].astype(BF16)  # [nb, T, D]
        xTc = np.concatenate([pack_pd(xb[i]) for i in range(len(bs))], axis=1)
        eb = e[bs].astype(BF16)  # [nb, S, D]
        eTc = pack_blocks(eb, blk, cfg)
        eT4c = pack_blocks(np.ascontiguousarray(eb[:, ::s4, :]), blk, cfg)
        bias = np.stack([bq[rows], bk[rows], bv[rows]]).reshape(6, 128).T
        wo_rows = Wo[:, rows].T.astype(BF16)  # [HP, D]
        woTc = wo_rows.reshape(2, 128, -1).transpose(1, 0, 2).reshape(128, -1)
        in_maps.append(
            {
                "xT": np.ascontiguousarray(xTc),
                "eT": np.ascontiguousarray(eTc),
                "eT4": np.ascontiguousarray(eT4c),
                "wqT": np.ascontiguousarray(pack_pd(Wq[rows].astype(BF16))),
                "wkT": np.ascontiguousarray(pack_pd(Wk[rows].astype(BF16))),
                "wvT": np.ascontiguousarray(pack_pd(Wv[rows].astype(BF16))),
                "woT": np.ascontiguousarray(woTc),
                "biases": np.ascontiguousarray(bias.astype(np.float32)),
            }
        )
    return in_maps


def gather_output(results, bo, cfg):
    Tl, Dl = cfg["T"], cfg["D"]
    out = np.zeros((B, Tl, Dl), np.float32)
    for c, (bs, _hs) in _core_map().items():
        p = results[c]["partial"].astype(np.float32).reshape(len(bs), Tl, Dl)
        for i, b in enumerate(bs):
            out[b] += p[i]
    return out + np.asarray(bo, np.float32)[None, None, :]


_COMPILED = None


def _get_compiled():
    global _COMPILED
    if _COMPILED is None:
        _COMPILED = build_program(FULL_CFG)
    return _COMPILED


def run_on_cores(inputs, trace=False, **kw):
    nc = _get_compiled()
    in_maps = shard_inputs(inputs, FULL_CFG)
    res = bass_utils.run_bass_kernel_spmd(
        nc, in_maps, core_ids=list(range(N_CORES)), trace=trace, **kw
    )
    return res


def kernel(**inputs) -> np.ndarray:
    res = run_on_cores(inputs, trace=False)
    return gather_output(res.results, inputs["bo"], FULL_CFG)



# revision 72
# speedup vs baseline: 1.0098x; 1.0098x over previous
# HEPOS cross-attention (strided per-head K/V) on 8 Trainium2 NeuronCores.
#
# Reference computation (per head h, stride s = STRIDE_LIST[h]):
#   Q = x @ Wq.T + bq ; K = e @ Wk.T + bk ; V = e @ Wv.T + bv
#   out_h = softmax(Q_h @ K_h[::s].T / 8) @ V_h[::s]
#   out   = concat_h(out_h) @ Wo.T + bo
#
# Sharding: 64 (batch, head) units over 8 cores. Core c owns head group
# g = c % 4 (heads 4g..4g+3, strides [1,2,4,8]) and batch pair [0,1]
# (c < 4) or [2,3] (c >= 4). Each core computes its heads' contribution
# to out; the host sums the four partials per batch and adds bo.
#
# On-device design (v3):
#  * Heads are processed as two stride PAIRS (sA, 2*sA): (1,2) and (4,8).
#    Head A of a pair lives on SBUF partitions 0-63, head B on 64-127.
#  * Q/K/V projections use the weight matrix as the matmul stationary with
#    both heads packed into the 128 stationary columns (full PE width).
#    K^T/V^T stream the "union" encoder columns (stride sA); head B rows
#    are valid at even union columns and are compacted on evacuation.
#    The stride-4 union for pair (4,8) is pre-packed by the host (eT4).
#  * V^T ([hd, keys]) is flipped to AV orientation ([keys, hd]) with ONE
#    DMA XBAR transpose per (block, head) (3D output access pattern) -
#    zero PE cost, one sync-engine dispatch each.
#  * Scores are computed transposed ([keys, T]); head B's score matmuls
#    use partitions 64-127 (PE row-tile T8) and overlap head A's (T0).
#  * AV accumulates into PSUM tiles resident across all encoder blocks of
#    a (batch, pair); the softmax denominator falls out of a ones-column
#    appended to the V stationary.
#  * Scores of chunk k+1 are issued before AV of chunk k so the PE never
#    waits on the scalar engine's exp.
#  * All DRAM->SBUF loads are single merged DMAs ([128, ndc, *] access
#    patterns); PSUM score tiles are bank-pair wide ([128, 2*tt]) so exp
#    and evacuations run as one instruction per tile.
#
# v4 (engine-FIFO / HAM-warmth pass; v3 measured ~150us of its 385us span
# at the 1.2 GHz throttled PE clock, re-triggered at every pair boundary):
#  * softmax normalize rebuilt: packed scalar+vector PSUM evacuations, one
#    reciprocal_approx_fast, K=2 selector-matmul partition broadcast into
#    the freed AV PSUM bank, one [128,tt] multiply per nt. No GpSimd, no
#    3.3us DVE reciprocal, vector-queue occupancy ~2.5us (was ~9us).
#  * exp merged to one ACT instruction per chunk ([128, 2tt]).
#  * out-proj evacuations alternate scalar/vector; store DMAs dispatch from
#    the gpsimd queue (sync queue head-of-line blocking broke et prefetch).
#  * startup: bias DMA early, warmup 40 x N=256 sized to the initial DMA
#    window, Q-proj double-buffered across the sc/kv PSUM pools.

import os
import sys

import ml_dtypes
import numpy as np

BF16 = ml_dtypes.bfloat16

for _p in ("/opt/trn_rl_repo", "/root/.axon_site/_ro/trn_rl_repo"):
    if os.path.isdir(_p) and _p not in sys.path:
        sys.path.insert(0, _p)

import concourse.bass as bass  # noqa: E402
import concourse.tile as tile  # noqa: E402
from concourse import bacc, mybir  # noqa: E402
from concourse import bass_utils  # noqa: E402

F32 = mybir.dt.float32
MM_DT = mybir.dt.bfloat16  # matmul operand dtype: full PE rate, half DMA
AF = mybir.ActivationFunctionType

D_MODEL = 1024
NUM_HEADS = 16
HEAD_DIM = 64
STRIDE_LIST = [1, 2, 4, 8] * 4
B, T, S = 4, 1024, 4096
N_CORES = 8

FULL_CFG = dict(
    nb=2,  # batches per core
    T=T,
    S=S,
    D=D_MODEL,
    strides=(1, 2, 4, 8),  # per-core head strides; pairs (s0,s1),(s2,s3)
    hd=HEAD_DIM,
    blk=1024,  # encoder S-block (stride-1 columns) per iteration
    tt=512,  # T tile (PSUM free-dim limit for fp32)
)

FR = MM_DT
WHOLE_BLOCK_TRANSPOSE = True


def _mm(nc, out, lhsT, rhs, start, stop):
    nc.tensor.matmul(out, lhsT, rhs, start=start, stop=stop)


def build_program(cfg):
    """Build the per-core Bass/Tile program (same program on all cores)."""
    nb, Tl, Sl, Dl = cfg["nb"], cfg["T"], cfg["S"], cfg["D"]
    strides, hd = cfg["strides"], cfg["hd"]
    assert strides[1] == 2 * strides[0] and strides[3] == 2 * strides[2]
    HP = 4 * hd  # packed head rows (256)
    s4 = strides[2]

    nc = bacc.Bacc(
        "TRN2",
        target_bir_lowering=False,
        debug=False,
        enable_asserts=False,
        num_devices=N_CORES,
    )

    # All inputs are HOST-PREPACKED to partition-major [128, ...] layouts so
    # every device DMA is a contiguous 2D slice (128 descriptors). The old
    # "(c p) u -> p c u" gather patterns cost ~1024 descriptors = 2-5.5us of
    # serial sync-engine dispatch PER LOAD, which dominated startup and
    # delayed et prefetches mid-stream.
    ndc_ = Dl // 128
    xT = nc.dram_tensor(
        "xT", [128, nb * ndc_ * Tl], MM_DT, kind="ExternalInput"
    ).ap()
    eT = nc.dram_tensor(
        "eT", [128, nb * ndc_ * Sl], MM_DT, kind="ExternalInput"
    ).ap()
    eT4 = nc.dram_tensor(
        "eT4", [128, nb * ndc_ * (Sl // s4)], MM_DT, kind="ExternalInput"
    ).ap()
    wqT = nc.dram_tensor("wqT", [128, ndc_ * HP], MM_DT, kind="ExternalInput").ap()
    wkT = nc.dram_tensor("wkT", [128, ndc_ * HP], MM_DT, kind="ExternalInput").ap()
    wvT = nc.dram_tensor("wvT", [128, ndc_ * HP], MM_DT, kind="ExternalInput").ap()
    woT = nc.dram_tensor("woT", [128, 2 * Dl], MM_DT, kind="ExternalInput").ap()
    biases = nc.dram_tensor("biases", [128, 6], F32, kind="ExternalInput").ap()
    out = nc.dram_tensor("partial", [nb * Tl, Dl], MM_DT, kind="ExternalOutput").ap()

    with tile.TileContext(nc) as tc:
        _build_tile(tc, cfg, xT, eT, eT4, wqT, wkT, wvT, woT, biases, out)

    nc.compile()
    return nc


def _build_tile(tc, cfg, xT, eT, eT4, wqT, wkT, wvT, woT, biases, out):
    nc = tc.nc
    nb, Tl, Sl, Dl = cfg["nb"], cfg["T"], cfg["S"], cfg["D"]
    strides, hd = cfg["strides"], cfg["hd"]
    blk, tt = cfg["blk"], cfg["tt"]
    ndc = Dl // 128
    nblk = Sl // blk
    ntt = Tl // tt
    assert ntt == 2, "wide PSUM tiles assume T == 2*tt"
    HP = 4 * hd
    scale = 1.0 / float(np.sqrt(hd))

    from contextlib import ExitStack

    with ExitStack() as ctx:
        wpool = ctx.enter_context(tc.tile_pool(name="weights", bufs=1))
        qtpool = ctx.enter_context(tc.tile_pool(name="qt", bufs=1))
        etpool = ctx.enter_context(tc.tile_pool(name="et", bufs=3))
        ktpool = ctx.enter_context(tc.tile_pool(name="kt", bufs=3))
        vtpool = ctx.enter_context(tc.tile_pool(name="vtT", bufs=2))
        vpool = ctx.enter_context(tc.tile_pool(name="v", bufs=3))
        ppool = ctx.enter_context(tc.tile_pool(name="p", bufs=4))
        npool = ctx.enter_context(tc.tile_pool(name="norm", bufs=2))
        otpool = ctx.enter_context(tc.tile_pool(name="ot", bufs=2))
        obpool = ctx.enter_context(tc.tile_pool(name="outs", bufs=3))
        # PSUM: ONE unified single-bank ring ([128, tt] fp32, bufs=4 -> 4
        # banks) shared by scores A/B, K/V projections, Q and out
        # projections. Each allocation only waits on the readers of the
        # allocation FOUR back, giving the score pipeline ~2 chunks of
        # slack over the exp stream - enough that transient ACT hiccups
        # never micro-stall the PE (micro-stalls oscillate the HAM clock
        # gate down to 1.2 GHz). av holds the 4 single-bank accumulators
        # -> 8 banks total.
        ps_ps = ctx.enter_context(tc.tile_pool(name="ps", bufs=4, space="PSUM"))
        av_ps = ctx.enter_context(tc.tile_pool(name="av_ps", bufs=1, space="PSUM"))

        def ps_tile():
            return ps_ps.tile([128, tt], F32, tag="ps", name="ps_psum")

        # ---- weights into SBUF (one DMA per tensor) ----
        wq_sb = wpool.tile([128, ndc * HP], FR, tag="wq", name="wq_sb")
        wk_sb = wpool.tile([128, ndc * HP], FR, tag="wk", name="wk_sb")
        wv_sb = wpool.tile([128, ndc * HP], FR, tag="wv", name="wv_sb")
        wo_sb = wpool.tile([128, 2 * Dl], FR, tag="wo", name="wo_sb")
        bias_sb = wpool.tile([128, 6], F32, tag="bias", name="bias_sb")
        # selector for the reciprocal partition-broadcast matmul:
        # rb[j, t] = rcp4[32 * (2nt + j // hd), t] via rb = sel.T @ rcp4-rows.
        # K=33 dense with live rows {0, 32} (legal start partitions); the
        # zero rows cancel den4's filler rows.
        sel_sb = wpool.tile([33, 128], FR, tag="sel", name="sel_sb")
        # rows {0, 32} = heads (legal start partitions), column halves = nt;
        # persistent + memset once so the filler rows stay initialized for
        # the whole-tile reciprocal.
        den4 = wpool.tile([33, 2 * tt], F32, tag="den4", name="den4")

        def wslice(wsb, dc, p):
            return wsb[:, dc * HP + p * 128 : dc * HP + (p + 1) * 128]

        # encoder block list + DMA helper (defined early so the first
        # block's load can be interleaved with the weight loads).
        # Blocks are uniform in UNION columns (blk per block) so pair (4,8)
        # gets one full-sized block instead of four tiny ones.
        def block_params(p):
            sA = strides[2 * p]
            Scols = Sl // sA
            return dict(
                src=eT if p == 0 else eT4,
                Scols=Scols,
                nblk_p=max(1, Scols // blk),
            )

        blocks = [
            (b, p, ib)
            for b in range(nb)
            for p in range(2)
            for ib in range(block_params(p)["nblk_p"])
        ]

        def block_ublk(p, ib):
            bp = block_params(p)
            return min(blk, bp["Scols"] - ib * blk)

        def emit_et_dma(b, p, ib):
            bp = block_params(p)
            ublk = block_ublk(p, ib)
            et = etpool.tile([128, ndc * ublk], FR, tag="et", name="et_t")
            base = (b * bp["nblk_p"] + ib) * ndc * ublk
            nc.sync.dma_start(out=et, in_=bp["src"][:, base : base + ndc * ublk])
            return et

        # ---- startup DMAs, ordered by first use: Q(b0) needs wq+xt0+bias,
        # then K0/V0 need wk/et0/wv, then et1 (phase_a(1) filler), then xt1
        # (deferred Q(b1) fillers), then wo (first out-proj, much later).
        # xt0 is split in dc-halves so Q(b0,p0)'s first dc-outer matmul
        # groups start after half the transfer.
        xpool = ctx.enter_context(tc.tile_pool(name="xt", bufs=1))
        xts = {}
        for b in range(nb):
            xts[b] = xpool.tile([128, ndc * Tl], FR, tag=f"xt{b}", name="xt")
        nc.sync.dma_start(out=wq_sb, in_=wqT)
        xh = (ndc // 2) * Tl
        nc.sync.dma_start(out=xts[0][:, 0:xh], in_=xT[:, 0:xh])
        nc.sync.dma_start(out=xts[0][:, xh : ndc * Tl], in_=xT[:, xh : ndc * Tl])
        nc.sync.dma_start(out=bias_sb, in_=biases)
        nc.sync.dma_start(out=wk_sb, in_=wkT)
        et_next = emit_et_dma(*blocks[0])
        nc.sync.dma_start(out=wv_sb, in_=wvT)
        et_next1 = emit_et_dma(*blocks[1]) if len(blocks) > 1 else None
        for b in range(1, nb):
            nc.sync.dma_start(
                out=xts[b],
                in_=xT[:, b * ndc * Tl : (b + 1) * ndc * Tl],
            )
        nc.sync.dma_start(out=wo_sb, in_=woT)
        nc.vector.memset(sel_sb, 0.0)
        nc.vector.memset(sel_sb[0:1, 0:hd], 1.0)
        nc.vector.memset(sel_sb[32:33, hd:128], 1.0)
        nc.vector.memset(den4, 1.0)

        # PE warm-up: dependency-free matmuls on a zeroed tile keep the
        # PE busy while the first DMAs land (~7us), so the HAM clock gate
        # opens (1.2 -> 2.4 GHz) before the real matmul stream begins.
        # 36 x N=256 at cold clock ~= 7.5us - sized to the DMA window.
        warm = wpool.tile([128, tt], FR, tag="warm", name="warm")
        nc.vector.memset(warm, 0.0)
        wps = ps_tile()
        for _ in range(36):
            _mm(
                nc, wps[:, 0:256], warm[:, 0:128], warm[:, 0:256],
                start=True, stop=True,
            )

        # ---- phase 1: Q^T = (x @ Wq.T + bq)^T, head pairs on partitions.
        # Only batch 0 is emitted up front; later batches become filler
        # items inside phase_b so they never block the tensor FIFO waiting
        # on their xt DMA.
        qt_sb = {}  # (b, pair) -> [128, T] tile

        def q_item(b, p):
            # dc-outer so the first matmul groups only need the first
            # dc-half of the xt DMA, and each stationary serves both nt.
            qt = qtpool.tile([128, Tl], FR, tag=f"qt{b}{p}", name="qt")
            qt_sb[(b, p)] = qt
            ps = [ps_tile() for _ in range(ntt)]
            for dc in range(ndc):
                for nt in range(ntt):
                    _mm(
                        nc,
                        ps[nt],
                        wslice(wq_sb, dc, p),
                        xts[b][:, dc * Tl + nt * tt : dc * Tl + (nt + 1) * tt],
                        start=(dc == 0),
                        stop=(dc == ndc - 1),
                    )
            for nt in range(ntt):
                nc.scalar.activation(
                    qt[:, nt * tt : (nt + 1) * tt],
                    ps[nt],
                    AF.Identity,
                    bias=bias_sb[:, p : p + 1],
                )

        for p in range(2):
            q_item(0, p)
        # bridge the Q-proj -> K-proj handoff: K0 waits on the et0 DMA
        # (~2.5us after Q drains); keep the PE busy so HAM stays warm.
        for _ in range(2):
            brg = ps_tile()
            for _ in range(4):
                _mm(
                    nc, brg[:, 0:256], warm[:, 0:128], warm[:, 0:256],
                    start=True, stop=True,
                )

        # ---- phase 2: attention per (batch, pair), out proj per batch ----
        # The per-block work is split into phase A (K^T/V^T projection,
        # evacuation, V transposes, next-block encoder DMA) and phase B
        # (scores/exp/AV chunk loop), software-pipelined one block deep:
        #   pA(0) pA(1) pB(0) pA(2) pB(1) ... pA(n-1) pB(n-3) pB(n-2) pB(n-1)
        # so V transposes are dispatched a full block before their AV
        # consumes them and the normalize chain never blocks evacuations.
        # AV emission inside phase B additionally lags scores by two chunk
        # steps so the PE never waits on the scalar engine's exp.
        assert Dl <= 2 * tt
        ot_sb = {}
        avp_live = {}
        blk_state = {}
        pending = []  # (age, avp, vt, pt, h, first, last)

        def flush_pending(min_age=2):
            keep = []
            for age, avp, vt, pt, h, first, last in pending:
                if age >= min_age:
                    for nt in range(ntt):
                        _mm(
                            nc,
                            avp[(h, nt)],
                            vt,
                            pt[:, nt * tt : (nt + 1) * tt],
                            start=first,
                            stop=last,
                        )
                else:
                    keep.append((age + 1, avp, vt, pt, h, first, last))
            pending[:] = keep

        def emit_norm(b, p):
            """Normalize the AV accumulators into ot. Designed to keep every
            engine FIFO short at pair boundaries (a clogged vector queue here
            stalls the next block's PSUM evacuations, idles the PE > 3.4us and
            drops the HAM clock gate to 1.2 GHz - the dominant cost in v3):
              * PSUM->SBUF evacuations split across scalar (h0) and vector
                (h1) queues, numerators packed on partitions 0:64 / 64:128.
              * reciprocal_approx_fast (one custom-DVE op, ~5x faster than
                reciprocal) on a [2, 2tt] tile holding all four denominators.
              * partition-broadcast of the reciprocals via a tiny K=2
                selector MATMUL into the just-freed AV PSUM bank (replaces
                two 1.1us GpSimd broadcast ping-pongs per (h,nt)).
              * one [128, tt] vector multiply per nt writes ot."""
            avp = avp_live.pop((b, p))
            rcp4 = npool.tile([33, 2 * tt], F32, tag="rcp4", name="rcp4")
            rcpb = npool.tile([33, 2 * tt], FR, tag="rcpb", name="rcpb")
            packed = {}
            for nt in range(ntt):
                pk = npool.tile([128, tt], F32, tag=f"pk{nt}", name="packed")
                packed[nt] = pk
                nc.scalar.copy(pk[0:hd, :], avp[(0, nt)][0:hd, :])
                nc.vector.tensor_copy(pk[hd : 2 * hd, :], avp[(1, nt)][0:hd, :])
                # den row copies cost a full free-dim pass each (DVE/ACT are
                # free-dim serial) - split them across the two engines
                nc.scalar.copy(
                    den4[0:1, nt * tt : (nt + 1) * tt],
                    avp[(0, nt)][hd : hd + 1, :],
                )
                nc.vector.tensor_copy(
                    den4[32:33, nt * tt : (nt + 1) * tt],
                    avp[(1, nt)][hd : hd + 1, :],
                )
            nc.vector.reciprocal_approx_fast(rcp4, den4)
            nc.scalar.copy(rcpb, rcp4)  # bf16 for the cheap selector matmul

            def part2():
                # the PE/vector half of the normalize, deferred a full block
                # (via the stash) so rcpb is long done when these drain as
                # filler items between score chunks.
                ot = otpool.tile([128, Tl], FR, tag=f"ot{p}", name="ot")
                ot_sb[(b, p)] = ot
                for nt in range(ntt):
                    rb = ps_tile()
                    _mm(
                        nc,
                        rb,
                        sel_sb,
                        rcpb[0:33, nt * tt : (nt + 1) * tt],
                        start=True,
                        stop=True,
                    )
                    nc.vector.tensor_mul(
                        ot[:, nt * tt : (nt + 1) * tt], packed[nt], rb
                    )

            return part2

        def out_proj_items(b):
            """One self-contained filler item per 128-row output tile:
            [ps alloc, 4 MMs, evacuation, store DMA]. Items are drained one
            per chunk inside phase_b so these exp-independent matmuls fill
            the PE's exp-wait bubbles instead of lumping at block edges."""

            def item(tc_i):
                ob = obpool.tile([128, Dl], FR, tag="ob", name="ob")
                for j in range(0, Dl, tt):
                    dw = min(tt, Dl - j)
                    ops = ps_tile()
                    for p in range(2):
                        _mm(
                            nc,
                            ops[:, 0:dw],
                            ot_sb[(b, p)][:, tc_i * 128 : (tc_i + 1) * 128],
                            wo_sb[:, p * Dl + j : p * Dl + j + dw],
                            start=(p == 0),
                            stop=(p == 1),
                        )
                    nc.vector.tensor_copy(ob[:, j : j + dw], ops[:, 0:dw])
                # the store DMA is dispatched from the (idle) gpsimd queue so
                # its input-ready wait never head-of-line blocks the sync
                # queue's et-prefetch / V-transpose dispatches.
                nc.gpsimd.dma_start(
                    out=out[b * Tl + tc_i * 128 : b * Tl + (tc_i + 1) * 128, :],
                    in_=ob,
                )

            import functools

            return [functools.partial(item, tc_i) for tc_i in range(Tl // 128)]

        ets = {0: et_next}
        if et_next1 is not None:
            ets[1] = et_next1

        def phase_a_items(bi):
            """The per-block K/V projection as self-contained per-tt-half
            filler items ([ps alloc, MMs, evacuations]) plus the V
            transposes / next-block DMA. Drained between phase_b chunks so
            the PE's exp-gated bubbles absorb the projection stream."""
            b, p, ib = blocks[bi]
            ublk = block_ublk(p, ib)
            nA = ublk // 128
            nB = nA // 2
            st = {}

            def proj_half(kind, c0):
                cw = min(tt, ublk - c0)
                if kind == "k":
                    if c0 == 0:
                        st["ktA"] = ktpool.tile(
                            [64, ublk], FR, tag="ktA", name="kt_A"
                        )
                        st["ktB"] = ktpool.tile(
                            [128, ublk // 2], FR, tag="ktB", name="kt_B"
                        )
                    dst_A, dst_B = st["ktA"], st["ktB"]
                    wsb, bcol = wk_sb, 2 + p
                else:
                    if c0 == 0:
                        st["vtA"] = vtpool.tile(
                            [64, ublk], FR, tag="vtA", name="vtT_A"
                        )
                        st["vtB"] = vtpool.tile(
                            [128, ublk // 2], FR, tag="vtB", name="vtT_B"
                        )
                    dst_A, dst_B = st["vtA"], st["vtB"]
                    wsb, bcol = wv_sb, 4 + p
                pps = ps_tile()
                et = ets[bi]
                for dc in range(ndc):
                    _mm(
                        nc,
                        pps[:, 0:cw],
                        wslice(wsb, dc, p),
                        et[:, dc * ublk + c0 : dc * ublk + c0 + cw],
                        start=(dc == 0),
                        stop=(dc == ndc - 1),
                    )
                nc.vector.tensor_scalar_add(
                    dst_A[:, c0 : c0 + cw],
                    pps[0:64, 0:cw],
                    bias_sb[0:64, bcol : bcol + 1],
                )
                nc.vector.tensor_scalar_add(
                    dst_B[64:128, c0 // 2 : (c0 + cw) // 2],
                    pps[64:128, 0:cw:2],
                    bias_sb[64:128, bcol : bcol + 1],
                )

            v_c0s = list(range(0, ublk, tt))

            def last_v_half():
                proj_half("v", v_c0s[-1])
                ets.pop(bi)
                vtT_A, vtT_B = st["vtA"], st["vtB"]

                # prefetch the NEXT block's encoder tile before the transposes
                if bi + 1 < len(blocks) and bi + 1 not in ets:
                    ets[bi + 1] = emit_et_dma(*blocks[bi + 1])

                # V -> [keys, hd] via DMA XBAR transposes - zero PE cost.
                # Chunk pitch 80 elems (160B) keeps destinations 32B-aligned.
                VP = hd + 16
                vtA = vpool.tile([128, nA * VP], FR, tag="vA", name="vtA")
                vtA3 = vtA.rearrange("p (c f) -> p c f", c=nA)
                nc.vector.memset(vtA3[:, :, hd : hd + 1], 1.0)
                vtB = vpool.tile([128, nB * VP], FR, tag="vB", name="vtB")
                vtB3 = vtB.rearrange("p (c f) -> p c f", c=nB)
                nc.vector.memset(vtB3[:, :, hd : hd + 1], 1.0)
                nc.sync.dma_start(out=vtA3[:, :, 0:hd], in_=vtT_A, transpose=True)
                nc.sync.dma_start(
                    out=vtB3[:, :, 0:hd], in_=vtT_B[64:128, :], transpose=True
                )
                blk_state[bi] = (st["ktA"], st["ktB"], vtA, vtB, nA, nB)

            import functools

            items = [
                functools.partial(proj_half, "k", c0)
                for c0 in range(0, ublk, tt)
            ]
            items += [
                functools.partial(proj_half, "v", c0) for c0 in v_c0s[:-1]
            ]
            items.append(last_v_half)
            return items

        def phase_b(bi, urgent, low):
            b, p, ib = blocks[bi]
            # safety: finish this block's phase_a / this batch's q_item
            while bi not in blk_state or (b, p) not in qt_sb:
                urgent.popleft()()
            kt_A, kt_B, vtA, vtB, nA, nB = blk_state.pop(bi)
            nblk_p = block_params(p)["nblk_p"]
            VP = hd + 16
            if (b, p) not in avp_live:
                avp_live[(b, p)] = {
                    (h, nt): av_ps.tile(
                        [hd + 1, tt], F32, tag=f"av{h}{nt}", name="av_psum"
                    )
                    for h in range(2)
                    for nt in range(ntt)
                }
            avp = avp_live[(b, p)]

            for ck in range(nA):
                do_B = ck % 2 == 1
                ckb = ck // 2
                ptA = ppool.tile([128, Tl], FR, tag="pA", name="ptA")
                sa = [ps_tile() for _ in range(ntt)]
                if do_B:
                    ptB = ppool.tile([128, Tl], FR, tag="pB", name="ptB")
                    sb_ = [ps_tile() for _ in range(ntt)]
                for nt in range(ntt):
                    _mm(
                        nc,
                        sa[nt],
                        kt_A[:, ck * 128 : (ck + 1) * 128],
                        qt_sb[(b, p)][0:64, nt * tt : (nt + 1) * tt],
                        start=True,
                        stop=True,
                    )
                    if do_B:
                        _mm(
                            nc,
                            sb_[nt],
                            kt_B[64:128, ckb * 128 : (ckb + 1) * 128],
                            qt_sb[(b, p)][64:128, nt * tt : (nt + 1) * tt],
                            start=True,
                            stop=True,
                        )
                # per-nt exps: each releases its single score bank as soon
                # as it completes, keeping the 4-deep ring flowing.
                for nt in range(ntt):
                    nc.scalar.activation(
                        ptA[:, nt * tt : (nt + 1) * tt], sa[nt], AF.Exp,
                        scale=scale,
                    )
                if do_B:
                    for nt in range(ntt):
                        nc.scalar.activation(
                            ptB[:, nt * tt : (nt + 1) * tt], sb_[nt], AF.Exp,
                            scale=scale,
                        )
                flush_pending()
                pending.append(
                    (
                        0,
                        avp,
                        vtA[:, ck * VP : ck * VP + hd + 1],
                        ptA,
                        0,
                        ib == 0 and ck == 0,
                        ib == nblk_p - 1 and ck == nA - 1,
                    )
                )
                if do_B:
                    pending.append(
                        (
                            0,
                            avp,
                            vtB[:, ckb * VP : ckb * VP + hd + 1],
                            ptB,
                            1,
                            ib == 0 and ckb == 0,
                            ib == nblk_p - 1 and ckb == nB - 1,
                        )
                    )
                # drain filler items: urgent (K/V projections - the et
                # prefetch chain depends on their progress) one per chunk;
                # deferrable work (out-proj, norm part2) only on odd chunks
                # so a reserve survives into the batch's last blocks.
                if urgent:
                    urgent.popleft()()
                elif low and (ck % 2 == 1 or bi >= nbl_holder[0] - 2):
                    low.popleft()()

            if ib == nblk_p - 1:
                flush_pending(min_age=0)
                norm_todo.append((b, p))

        from collections import deque

        nbl = len(blocks)
        nbl_holder = [nbl]
        norm_todo = []
        urgent = deque()
        low = deque()

        def dummy_item():
            # dependency-free warm-keeper matmuls: bridge the sparse
            # exp-paced stretches of the last blocks / final normalize so
            # the HAM activity window never re-throttles the PE to 1.2 GHz.
            dp = ps_tile()
            for _ in range(4):
                _mm(
                    nc, dp[:, 0:256], warm[:, 0:128], warm[:, 0:256],
                    start=True, stop=True,
                )
        for it in phase_a_items(0):
            it()
        if nbl > 1:
            urgent.extend(phase_a_items(1))
        for b in range(1, nb):
            urgent.extend(
                (lambda b_, p_: lambda: q_item(b_, p_))(b, p) for p in range(2)
            )
        stash = []  # norm part2 items, delayed one block so their inputs
        # (the pair-end reciprocal chain) are long done when they drain
        for i in range(nbl):
            if i + 2 < nbl:
                urgent.extend(phase_a_items(i + 2))
            b_i, p_i, ib_i = blocks[i]
            if b_i > 0 and p_i == 0 and ib_i == 0:
                # previous batch's out projection, deferred past its norm
                # (its emission must follow the stashed norm part2)
                low.extend(stash)
                stash.clear()
                low.extend(out_proj_items(b_i - 1))
            if i >= nbl - 2:
                low.extend([dummy_item] * 4)
            # norm part2 after the dummies: its rb matmul waits on the
            # pair-end reciprocal/cast chain, so give it extra runway
            low.extend(stash)
            stash.clear()
            phase_b(i, urgent, low)
            while norm_todo:
                nb_, np_ = norm_todo.pop(0)
                stash.append(emit_norm(nb_, np_))
        low.extend([dummy_item] * 7)
        low.extend(stash)
        stash.clear()
        while urgent:
            urgent.popleft()()
        while low:
            low.popleft()()
        for it in out_proj_items(nb - 1):
            it()

# ---------------------------------------------------------------------------
# Host-side sharding / gathering
# ---------------------------------------------------------------------------


def _core_map():
    """core -> (batches, heads)"""
    m = {}
    for c in range(N_CORES):
        g = c % 4
        bs = [0, 1] if c < 4 else [2, 3]
        hs = [4 * g + i for i in range(4)]
        m[c] = (bs, hs)
    return m


def pack_pd(a2d):
    """[N, D] -> [128, ndc * N] partition-major: out[p, dc*N + n] =
    a2d[n, dc*128 + p]. Makes the device DMA a contiguous 2D slice."""
    N, D = a2d.shape
    ndc = D // 128
    return (
        a2d.reshape(N, ndc, 128).transpose(2, 1, 0).reshape(128, ndc * N)
    )


def pack_blocks(e_b, blk, cfg):
    """Per-batch encoder rows [nb, Scols, D] -> [128, nb*nblk*ndc*ublk]
    with each (b, ib) block pack_pd'd and laid out contiguously."""
    nbb, Scols, D = e_b.shape
    nblk = max(1, Scols // blk)
    ublk = Scols // nblk
    parts = [
        pack_pd(e_b[b, ib * ublk : (ib + 1) * ublk])
        for b in range(nbb)
        for ib in range(nblk)
    ]
    return np.concatenate(parts, axis=1)


def shard_inputs(inputs, cfg):
    x = np.asarray(inputs["decoder_input"], np.float32)
    e = np.asarray(inputs["encoder_output"], np.float32)
    Wq = np.asarray(inputs["Wq"], np.float32)
    Wk = np.asarray(inputs["Wk"], np.float32)
    Wv = np.asarray(inputs["Wv"], np.float32)
    Wo = np.asarray(inputs["Wo"], np.float32)
    bq = np.asarray(inputs["bq"], np.float32)
    bk = np.asarray(inputs["bk"], np.float32)
    bv = np.asarray(inputs["bv"], np.float32)
    hd = cfg["hd"]
    s4 = cfg["strides"][2]
    blk = cfg["blk"]
    in_maps = []
    for c, (bs, hs) in _core_map().items():
        rows = np.concatenate([np.arange(h * hd, (h + 1) * hd) for h in hs])
        xb = x[bs].astype(BF16)  # [nb, T, D]
        xTc = np.concatenate([pack_pd(xb[i]) for i in range(len(bs))], axis=1)
        eb = e[bs].astype(BF16)  # [nb, S, D]
        eTc = pack_blocks(eb, blk, cfg)
        eT4c = pack_blocks(np.ascontiguousarray(eb[:, ::s4, :]), blk, cfg)
        bias = np.stack([bq[rows], bk[rows], bv[rows]]).reshape(6, 128).T
        wo_rows = Wo[:, rows].T.astype(BF16)  # [HP, D]
        woTc = wo_rows.reshape(2, 128, -1).transpose(1, 0, 2).reshape(128, -1)
        in_maps.append(
            {
                "xT": np.ascontiguousarray(xTc),
                "eT": np.ascontiguousarray(eTc),
                "eT4": np.ascontiguousarray(eT4c),
                "wqT": np.ascontiguousarray(pack_pd(Wq[rows].astype(BF16))),
                "wkT": np.ascontiguousarray(pack_pd(Wk[rows].astype(BF16))),
                "wvT": np.ascontiguousarray(pack_pd(Wv[rows].astype(BF16))),
                "woT": np.ascontiguousarray(woTc),
                "biases": np.ascontiguousarray(bias.astype(np.float32)),
            }
        )
    return in_maps


def gather_output(results, bo, cfg):
    Tl, Dl = cfg["T"], cfg["D"]
    out = np.zeros((B, Tl, Dl), np.float32)
    for c, (bs, _hs) in _core_map().items():
        p = results[c]["partial"].astype(np.float32).reshape(len(bs), Tl, Dl)
        for i, b in enumerate(bs):
            out[b] += p[i]
    return out + np.asarray(bo, np.float32)[None, None, :]


_COMPILED = None


def _get_compiled():
    global _COMPILED
    if _COMPILED is None:
        _COMPILED = build_program(FULL_CFG)
    return _COMPILED


def run_on_cores(inputs, trace=False, **kw):
    nc = _get_compiled()
    in_maps = shard_inputs(inputs, FULL_CFG)
    res = bass_utils.run_bass_kernel_spmd(
        nc, in_maps, core_ids=list(range(N_CORES)), trace=trace, **kw
    )
    return res


def kernel(**inputs) -> np.ndarray:
    res = run_on_cores(inputs, trace=False)
    return gather_output(res.results, inputs["bo"], FULL_CFG)

